# revision 5
# baseline (speedup 1.0000x reference)
"""MixerDiffAttention Trainium2 kernel.

Sharding: 8 cores = batch(2) x head-group(4).  Core (b, r) computes output
heads {2r, 2r+1} of batch b: q-heads {2r,2r+1,8+2r,8+2r+1}, k-heads {r, 4+r},
v-head r.  Inside: fused causal pipeline over 512-token chunks — project,
RMS+RoPE+scale in natural layout, PE-transpose to q^T/k^T, then attention as
S^T = K Q^T (max-free softmax, ones-column on V gives the denominator),
differential combine + RMS + gn.  Matmuls in float32r (TF32-like, full rate).
"""
import numpy as np
import concourse.bass as bass
import concourse.mybir as mybir
from concourse import bacc
from concourse.tile import TileContext
from concourse.bass_utils import run_bass_kernel_spmd

B, T, DM = 2, 2048, 2048
H, KVH, D = 16, 8, 128
TC = 512                  # token chunk (= q chunk)
NT = T // 128             # 16 token tiles
NCH = T // TC             # 4 chunks
NDM = DM // 128           # 16 contraction chunks
EPS = 1e-6
ROPE_BASE = 10000.0
LAMBDA_INIT = 0.8 - 0.6 * np.exp(-0.3 * 12)
F32 = mybir.dt.float32
MM_DT = mybir.dt.float32r
AF = mybir.ActivationFunctionType
ALU = mybir.AluOpType
ISQ = float(1.0 / np.sqrt(D))
MASK_NEG = -1e30


def _bc_mid(a, n):
    # [128, m] AP -> [128, n(bcast), m]
    return bass.AP(tensor=a.tensor, offset=a.offset, ap=[list(a.ap[0]), [0, n], list(a.ap[1])])


def _bc_last(a, n):
    # [128, m] AP -> [128, m, n(bcast)]
    return bass.AP(tensor=a.tensor, offset=a.offset, ap=[list(a.ap[0]), list(a.ap[1]), [0, n]])


def _build():
    nc = bacc.Bacc(None, target_bir_lowering=False)

    xT = nc.dram_tensor("xT", [DM, T], F32, kind="ExternalInput")
    wq_d = nc.dram_tensor("wq", [DM, 512], F32, kind="ExternalInput")
    wkv_d = nc.dram_tensor("wkv", [DM, 512], F32, kind="ExternalInput")
    scal_d = nc.dram_tensor("scal", [128, 4], F32, kind="ExternalInput")
    gn_d = nc.dram_tensor("gnw", [128, 2, 256], F32, kind="ExternalInput")
    neglam_d = nc.dram_tensor("neglam", [128, 1], F32, kind="ExternalInput")
    y_d = nc.dram_tensor("y", [T, 2, 256], F32, kind="ExternalOutput")

    # constant tables, laid out [128 partitions, NT tiles, ...] host-side
    pos = np.arange(T, dtype=np.float64)
    inv = ROPE_BASE ** (-np.arange(0, D, 2, dtype=np.float64) / D)  # (64,)
    ang = np.outer(pos, inv)
    cos_h = np.cos(ang).astype(np.float32).reshape(NT, 128, 64).transpose(1, 0, 2).copy()
    sin_h = np.sin(ang).astype(np.float32).reshape(NT, 128, 64).transpose(1, 0, 2).copy()
    logp_h = np.log(np.arange(1, T + 1, dtype=np.float64)).astype(np.float32)
    logp_h = logp_h.reshape(NT, 128, 1).transpose(1, 0, 2).copy()
    # wide causal mask: mask_j[p, f] = NEG if p + 128j > f ; mask_j = wide[:, f + (3-j)*128]
    pidx = np.arange(128).reshape(128, 1)
    g = np.arange(TC + 384).reshape(1, TC + 384)
    wide_h = np.where(pidx + 384 > g, np.float32(MASK_NEG), np.float32(0.0)).astype(np.float32)
    ident_h = np.eye(128, dtype=np.float32)

    cos_c = nc.inline_tensor(cos_h, "cos_c")
    sin_c = nc.inline_tensor(sin_h, "sin_c")
    logp_c = nc.inline_tensor(logp_h, "logp_c")
    wide_c = nc.inline_tensor(wide_h, "wide_c")
    ident_c = nc.inline_tensor(ident_h, "ident_c")

    with TileContext(nc) as tc:
        with (
            tc.tile_pool(name="wp", bufs=1) as wp,
            tc.tile_pool(name="cp", bufs=1) as cp,
            tc.tile_pool(name="xp", bufs=17) as xp,
            tc.tile_pool(name="kv", bufs=1) as kvp,
            tc.tile_pool(name="qt", bufs=2) as qtp,
            tc.tile_pool(name="wk", bufs=2) as wk,
            tc.tile_pool(name="pt", bufs=3) as ptp,
            tc.tile_pool(name="yo", bufs=4) as yop,
            tc.tile_pool(name="ps_p", bufs=2, space="PSUM") as ps_p,
            tc.tile_pool(name="ps_s", bufs=2, space="PSUM") as ps_s,
            tc.tile_pool(name="ps_o", bufs=4, space="PSUM") as ps_o,
        ):
            # ---- persistent loads ----
            wq_sb = wp.tile([128, NDM, 512], MM_DT, tag="wq")
            wkv_sb = wp.tile([128, NDM, 512], MM_DT, tag="wkv")
            nc.gpsimd.dma_start(out=wq_sb, in_=wq_d.ap().rearrange("(n p) m -> p n m", p=128))
            nc.gpsimd.dma_start(out=wkv_sb, in_=wkv_d.ap().rearrange("(n p) m -> p n m", p=128))

            cos_sb = cp.tile([128, NT, 64], F32, tag="cos")
            sin_sb = cp.tile([128, NT, 64], F32, tag="sin")
            logp_sb = cp.tile([128, NT, 1], F32, tag="logp")
            wide_sb = cp.tile([128, TC + 384], F32, tag="wide")
            ident_sb = cp.tile([128, 128], F32, tag="ident")
            scal_sb = cp.tile([128, 4], F32, tag="scal")
            gn_sb = cp.tile([128, 2, 256], F32, tag="gn")
            neglam_sb = cp.tile([128, 1], F32, tag="neglam")
            eps_sb = cp.tile([128, 1], F32, tag="eps")
            nc.sync.dma_start(out=cos_sb, in_=cos_c.ap())
            nc.sync.dma_start(out=sin_sb, in_=sin_c.ap())
            nc.sync.dma_start(out=logp_sb, in_=logp_c.ap())
            nc.sync.dma_start(out=wide_sb, in_=wide_c.ap())
            nc.sync.dma_start(out=ident_sb, in_=ident_c.ap())
            nc.sync.dma_start(out=scal_sb, in_=scal_d.ap())
            nc.sync.dma_start(out=gn_sb, in_=gn_d.ap())
            nc.sync.dma_start(out=neglam_sb, in_=neglam_d.ap())
            nc.vector.memset(eps_sb[:], EPS)

            # per-token-tile persistent K^T and V(+ones)
            kT_t = [kvp.tile([128, 2, 128], MM_DT, tag=f"kT{i}", name=f"kT{i}") for i in range(NT)]
            vA_t = [kvp.tile([128, 258], MM_DT, tag=f"vA{i}", name=f"vA{i}") for i in range(NT)]
            for i in range(NT):
                nc.vector.memset(vA_t[i][:, 256:258].bitcast(F32), 1.0)

            for c in range(NCH):
                # ---- load x^T chunk (cast to f32r) ----
                xts = []
                for dmi in range(NDM):
                    xt_t = xp.tile([128, TC], MM_DT, tag="xt")
                    nc.gpsimd.dma_start(
                        out=xt_t,
                        in_=xT.ap()[dmi * 128:(dmi + 1) * 128, c * TC:(c + 1) * TC],
                    )
                    xts.append(xt_t)

                qT_ch = qtp.tile([128, 4, TC], MM_DT, tag="qtc")

                for ti in range(4):
                    tt = c * 4 + ti
                    # ---- projections ----
                    q_ps = ps_p.tile([128, 512], F32, tag="pp")
                    kv_ps = ps_p.tile([128, 512], F32, tag="pp")
                    for dmi in range(NDM):
                        lhs = xts[dmi][:, ti * 128:(ti + 1) * 128]
                        nc.tensor.matmul(q_ps[:], lhs, wq_sb[:, dmi, :],
                                         start=(dmi == 0), stop=(dmi == NDM - 1))
                        nc.tensor.matmul(kv_ps[:], lhs, wkv_sb[:, dmi, :],
                                         start=(dmi == 0), stop=(dmi == NDM - 1))

                    # ---- q processing: copy, rms stats, scale, rope ----
                    q_sb = wk.tile([128, 512], F32, tag="q")
                    nc.scalar.copy(out=q_sb[:], in_=q_ps[:])
                    qr = wk.tile([128, 512], F32, tag="qr")       # scratch then rotated q
                    ssq = wk.tile([128, 4], F32, tag="ssq")
                    for h in range(4):
                        nc.scalar.activation(out=qr[:, h * 128:(h + 1) * 128],
                                             in_=q_ps[:, h * 128:(h + 1) * 128],
                                             func=AF.Square, accum_out=ssq[:, h:h + 1])
                    rsq = wk.tile([128, 4], F32, tag="rsq")
                    nc.scalar.activation(out=rsq[:], in_=ssq[:], func=AF.Sqrt,
                                         scale=1.0 / D, bias=eps_sb[:])
                    nc.vector.reciprocal(rsq[:], rsq[:])
                    nc.vector.tensor_scalar_mul(rsq[:], rsq[:], logp_sb[:, tt, :])
                    nc.vector.tensor_mul(rsq[:], rsq[:], scal_sb[:])
                    qv = q_sb[:].rearrange("p (h d) -> p h d", h=4)
                    nc.vector.tensor_mul(qv, qv, _bc_last(rsq[:], 128))
                    # rope
                    qrv = qr[:].rearrange("p (h d) -> p h d", h=4)
                    cos4 = _bc_mid(cos_sb[:, tt, :], 4)
                    sin4 = _bc_mid(sin_sb[:, tt, :], 4)
                    t1 = wk.tile([128, 4, 64], F32, tag="t1")
                    nc.vector.tensor_mul(qrv[:, :, 0:64], qv[:, :, 0:64], cos4)
                    nc.vector.tensor_mul(t1[:], qv[:, :, 64:128], sin4)
                    nc.vector.tensor_add(qrv[:, :, 0:64], qrv[:, :, 0:64], t1[:])
                    nc.vector.tensor_mul(qrv[:, :, 64:128], qv[:, :, 64:128], cos4)
                    nc.vector.tensor_mul(t1[:], qv[:, :, 0:64], sin4)
                    nc.vector.tensor_sub(qrv[:, :, 64:128], qrv[:, :, 64:128], t1[:])

                    # ---- k processing ----
                    k_sb = wk.tile([128, 256], F32, tag="k")
                    nc.scalar.copy(out=k_sb[:], in_=kv_ps[:, 0:256])
                    kr = wk.tile([128, 256], F32, tag="kr")
                    ssk = wk.tile([128, 2], F32, tag="ssk")
                    for h in range(2):
                        nc.scalar.activation(out=kr[:, h * 128:(h + 1) * 128],
                                             in_=kv_ps[:, h * 128:(h + 1) * 128],
                                             func=AF.Square, accum_out=ssk[:, h:h + 1])
                    rsk = wk.tile([128, 2], F32, tag="rsk")
                    nc.scalar.activation(out=rsk[:], in_=ssk[:], func=AF.Sqrt,
                                         scale=1.0 / D, bias=eps_sb[:])
                    nc.vector.reciprocal(rsk[:], rsk[:])
                    kv_ = k_sb[:].rearrange("p (h d) -> p h d", h=2)
                    nc.vector.tensor_mul(kv_, kv_, _bc_last(rsk[:], 128))
                    krv = kr[:].rearrange("p (h d) -> p h d", h=2)
                    cos2 = _bc_mid(cos_sb[:, tt, :], 2)
                    sin2 = _bc_mid(sin_sb[:, tt, :], 2)
                    t2 = wk.tile([128, 2, 64], F32, tag="t2")
                    nc.vector.tensor_mul(krv[:, :, 0:64], kv_[:, :, 0:64], cos2)
                    nc.vector.tensor_mul(t2[:], kv_[:, :, 64:128], sin2)
                    nc.vector.tensor_add(krv[:, :, 0:64], krv[:, :, 0:64], t2[:])
                    nc.vector.tensor_mul(krv[:, :, 64:128], kv_[:, :, 64:128], cos2)
                    nc.vector.tensor_mul(t2[:], kv_[:, :, 0:64], sin2)
                    nc.vector.tensor_sub(krv[:, :, 64:128], krv[:, :, 64:128], t2[:])

                    # ---- v (+ ones col already set) ----
                    nc.vector.tensor_copy(out=vA_t[tt][:, 0:256], in_=kv_ps[:, 256:512])

                    # ---- transposes ----
                    for h in range(4):
                        tp = ps_s.tile([128, 128], F32, tag="st")
                        nc.tensor.transpose(tp[:], qr[:, h * 128:(h + 1) * 128], ident_sb[:])
                        nc.any.tensor_copy(out=qT_ch[:, h, ti * 128:(ti + 1) * 128], in_=tp[:])
                    for h in range(2):
                        tp = ps_s.tile([128, 128], F32, tag="st")
                        nc.tensor.transpose(tp[:], kr[:, h * 128:(h + 1) * 128], ident_sb[:])
                        nc.any.tensor_copy(out=kT_t[tt][:, h, :], in_=tp[:])

                # ---- attention for q-chunk c ----
                for h in range(2):
                    y1 = wk.tile([128, 4, 256], F32, tag="y1")
                    for s in range(2):
                        o_t = [ps_o.tile([128, 258], F32, tag="o", name=f"o{_sq}") for _sq in range(4)]
                        for kt in range(4 * (c + 1)):
                            st = ps_s.tile([128, 512], F32, tag="st")
                            nc.tensor.matmul(st[:], kT_t[kt][:, s, :], qT_ch[:, 2 * s + h, :],
                                             start=True, stop=True)
                            j = kt - 4 * c
                            if j >= 0:
                                off = (3 - j) * 128
                                nc.vector.tensor_add(st[:], st[:], wide_sb[:, off:off + TC])
                            pt = ptp.tile([128, 512], MM_DT, tag="pt")
                            nc.scalar.activation(out=pt[:], in_=st[:], func=AF.Exp, scale=ISQ)
                            for sq in range(4):
                                qt_g = 4 * c + sq
                                if qt_g < kt:
                                    continue
                                nc.tensor.matmul(o_t[sq][:], pt[:, sq * 128:(sq + 1) * 128],
                                                 vA_t[kt][:], start=(kt == 0), stop=(kt == qt_g))
                        for sq in range(4):
                            ot = o_t[sq]
                            rec = wk.tile([128, 1], F32, tag="rec")
                            nc.vector.reciprocal(rec[:], ot[:, 256:257])
                            if s == 0:
                                nc.vector.tensor_scalar_mul(y1[:, sq, :], ot[:, 0:256], rec[:])
                            else:
                                nc.vector.tensor_mul(rec[:], rec[:], neglam_sb[:])
                                yv = wk.tile([128, 256], F32, tag="yv")
                                nc.vector.scalar_tensor_tensor(
                                    out=yv[:], in0=ot[:, 0:256], scalar=rec[:],
                                    in1=y1[:, sq, :], op0=ALU.mult, op1=ALU.add)
                                s2 = wk.tile([128, 1], F32, tag="s2")
                                sq2 = wk.tile([128, 256], F32, tag="sq2")
                                nc.scalar.activation(out=sq2[:], in_=yv[:], func=AF.Square,
                                                     accum_out=s2[:])
                                rs = wk.tile([128, 1], F32, tag="rs")
                                nc.scalar.activation(out=rs[:], in_=s2[:], func=AF.Sqrt,
                                                     scale=1.0 / 256, bias=eps_sb[:])
                                nc.vector.reciprocal(rs[:], rs[:])
                                yo = yop.tile([128, 256], F32, tag="yo")
                                nc.vector.scalar_tensor_tensor(
                                    out=yo[:], in0=yv[:], scalar=rs[:],
                                    in1=gn_sb[:, h, :], op0=ALU.mult, op1=ALU.mult)
                                qt_g = 4 * c + sq
                                nc.sync.dma_start(
                                    out=y_d.ap()[qt_g * 128:(qt_g + 1) * 128, h, :],
                                    in_=yo[:])
    nc.compile()
    return nc


_NC = None
_last_in_maps = None


def _get_nc():
    global _NC
    if _NC is None:
        _NC = _build()
    return _NC


def kernel(x, Wq, Wk, Wv, lambda_q1, lambda_k1, lambda_q2, lambda_k2,
           softmax_scaler, gn_weight):
    x = np.asarray(x, np.float32)
    Wq = np.asarray(Wq, np.float32)
    Wk = np.asarray(Wk, np.float32)
    Wv = np.asarray(Wv, np.float32)
    lam = float(np.exp(np.sum(np.float64(lambda_q1) * np.float64(lambda_k1)))
                - np.exp(np.sum(np.float64(lambda_q2) * np.float64(lambda_k2)))
                + LAMBDA_INIT)
    softmax_scaler = np.asarray(softmax_scaler, np.float32)
    gn_weight = np.asarray(gn_weight, np.float32)

    nc = _get_nc()
    in_maps = []
    for core in range(8):
        b, r = divmod(core, 4)
        qheads = [2 * r, 2 * r + 1, 8 + 2 * r, 8 + 2 * r + 1]
        wq_c = np.concatenate([Wq[:, hh * 128:(hh + 1) * 128] for hh in qheads], axis=1)
        wkv_c = np.concatenate([
            Wk[:, r * 128:(r + 1) * 128],
            Wk[:, (4 + r) * 128:(5 + r) * 128],
            Wv[:, r * 256:(r + 1) * 256],
        ], axis=1)
        in_maps.append({
            "xT": np.ascontiguousarray(x[b].T),
            "wq": np.ascontiguousarray(wq_c),
            "wkv": np.ascontiguousarray(wkv_c),
            "scal": np.ascontiguousarray(
                np.broadcast_to(softmax_scaler[qheads].reshape(1, 4), (128, 4))),
            "gnw": np.ascontiguousarray(
                np.broadcast_to(gn_weight[2 * r:2 * r + 2].reshape(1, 2, 256), (128, 2, 256))),
            "neglam": np.full((128, 1), -lam, np.float32),
        })
    global _last_in_maps
    _last_in_maps = in_maps
    res = run_bass_kernel_spmd(nc, in_maps, list(range(8)))
    out = np.empty((B, T, 8, 256), np.float32)
    for core in range(8):
        b, r = divmod(core, 4)
        out[b, :, 2 * r:2 * r + 2, :] = res.results[core]["y"]
    return out
